# revision 2
# baseline (speedup 1.0000x reference)
"""Trainium2 Bass kernel for BaseTopoLayer GNN message passing (v2).

Node-partitioned across 8 cores (each core owns all edges whose dst lands in
its node blocks; softmax segments fully local, no collectives).

v2 dataflow ("transposed hybrid"), per 512-edge group:
- First MLP layer runs channel-on-partition (transposed): pre1T[c,e] built
  from wide N=512 matmuls with the W1 chunks as stationary lhsT.
  h[dst]/q[dst] gathers are matmuls against a host-built selection matrix ST;
  h[src] rows are indirect-DMA gathered then PE-transposed.
- LayerNorm variance = ones-vector matmul over partitions (PE), so no ACT
  Square-accum per tile; rstd = exp(-.5 ln(var/D+eps)) stays in the single
  exp/ln/square/copy activation-table set.
- Scores: prodT = kT * qdstT elementwise, then a head-indicator matmul
  (1/sqrt(hd) folded in) reduces within heads; scores+vars ride one PSUM bank
  and are transposed back to edge-partition orientation together.
- Second layer consumes hreluT directly as lhsT (no transposes), producing
  k in transposed and v in normal orientation; softmax scales fold into the
  16 exp values per edge, not the 128 v values.
- Scatter per subtile is one matmul with the host-built S (= ST^T).
"""

import numpy as np
import ml_dtypes

import concourse.bass as bass
import concourse.mybir as mybir
from concourse.tile import TileContext
from concourse.vector_clock import ScopedClock
from concourse.bass_utils import run_bass_kernel_spmd
from concourse.masks import make_identity

BF16 = mybir.dt.bfloat16
F32 = mybir.dt.float32
I32 = mybir.dt.int32
AF = mybir.ActivationFunctionType
ALU = mybir.AluOpType

NCORES = 8
P = 128
HEADS = 16
EPS = 1e-5
G = 4  # subtiles (128 edges) per group


# ---------------------------------------------------------------------------
# Tile drain patch: this neuronxcc build rejects >N sem waits on one Drain.
def _patched_drain(self, tick_clock, wait_clock):
    nc = self.nc
    drain_inst = nc.sync.drain()
    wait_clock.add_sem_waits(
        drain_inst.ins, ScopedClock({None: tick_clock.global_clock})
    )
    si = drain_inst.ins.sync_info
    waits = list(si.on_wait or [])
    if len(waits) > 1:
        si.on_wait = [waits[0]]
        for w in waits[1:]:
            nop = nc.sync.nop(nofuse=True)
            nop.ins.sync_info = mybir.SyncInfo(on_wait=[w], on_update=[])
    nc.all_engine_barrier()
    assert self.sems is not None
    popped = nc._tile_sem_poison_stack.pop()
    assert popped is self._sem_poison
    nc.clear_and_free_semaphores(list(self.sems.allocated().values()))
    nc.all_engine_barrier()


TileContext._drain_and_barrier = _patched_drain


def _split_excess_waits(nc, max_waits=1):
    """Move excess sem waits onto same-engine nops placed just before."""
    cnt = 0
    for bb in nc.main_func.blocks:
        newlist = []
        for inst in bb.instructions:
            si = inst.sync_info
            waits = list(si.on_wait) if si is not None and si.on_wait else []
            if len(waits) > max_waits:
                si.on_wait = waits[:max_waits]
                for w in waits[max_waits:]:
                    nop = mybir.InstNoOp(name=f"waitnop-{cnt}", ins=[], outs=[])
                    cnt += 1
                    nop.engine = inst.engine
                    nop.sync_info = mybir.SyncInfo(on_wait=[w], on_update=[])
                    newlist.append(nop)
            newlist.append(inst)
        bb.instructions = newlist
    return cnt


def _bf(x):
    return np.ascontiguousarray(np.asarray(x, np.float32).astype(ml_dtypes.bfloat16))


def _f32(x):
    return np.ascontiguousarray(np.asarray(x, np.float32))


# ---------------------------------------------------------------------------
# Host-side partitioning: nodes -> (core, block, slot) with edge balancing.
def _partition(dst, N, B):
    import heapq

    G_ = NCORES * B
    deg = np.bincount(dst, minlength=N)
    order = np.argsort(-deg, kind="stable")
    heap = [(0, 0, g) for g in range(G_)]
    heapq.heapify(heap)
    gblock_of = np.empty(N, np.int32)
    slot_of = np.empty(N, np.int32)
    stash = []
    for n in order:
        while True:
            load, cnt, g = heapq.heappop(heap)
            if cnt < P:
                break
            stash.append((load, cnt, g))
        gblock_of[n] = g
        slot_of[n] = cnt
        heapq.heappush(heap, (load + int(deg[n]), cnt + 1, g))
        for s in stash:
            heapq.heappush(heap, s)
        stash.clear()
    loads = np.bincount(gblock_of, weights=deg, minlength=G_).astype(np.int64)
    order_g = np.argsort(-loads, kind="stable")
    core_of_g = np.empty(G_, np.int32)
    lblock_of_g = np.empty(G_, np.int32)
    core_loads = [(0.0, c) for c in range(NCORES)]
    heapq.heapify(core_loads)
    core_fill = [0] * NCORES
    for g in order_g:
        while True:
            cl, c = heapq.heappop(core_loads)
            if core_fill[c] < B:
                break
        core_of_g[g] = c
        lblock_of_g[g] = core_fill[c]
        core_fill[c] += 1
        heapq.heappush(core_loads, (cl + loads[g], c))
    return gblock_of, slot_of, core_of_g, lblock_of_g


# ---------------------------------------------------------------------------
def _prep(inputs):
    """All host-side preprocessing. Returns (meta, in_maps)."""
    h = _f32(inputs["h"])
    r_feat = _f32(inputs["r_feat"])
    edge_feat = _f32(inputs["edge_feat"])
    e_w = _f32(inputs["e_w"])
    ei = np.asarray(inputs["edge_index"])
    src = ei[0].astype(np.int64)
    dst = ei[1].astype(np.int64)

    N, D = h.shape
    E = src.shape[0]
    hd = D // HEADS
    assert D == 128, "kernel assumes D=128"

    def center(W1, b1):
        W1 = _f32(W1)
        b1 = _f32(b1)
        return W1 - W1.mean(axis=1, keepdims=True), b1 - b1.mean()

    w1k, b1k = center(inputs["xk_W1"], inputs["xk_b1"])
    w1v, b1v = center(inputs["xv_W1"], inputs["xv_b1"])
    w1q, b1q = center(inputs["xq_W1"], inputs["xq_b1"])
    w1o, b1o = center(inputs["out_W1"], inputs["out_b1"])
    assert np.allclose(b1q, 0.0) and np.allclose(b1o, 0.0), "nonzero b1 in q/out MLP"

    for m in ("xk", "xv", "xq", "out"):
        g = _f32(inputs[f"{m}_g"])
        be = _f32(inputs[f"{m}_beta"])
        b2 = _f32(inputs[f"{m}_b2"])
        assert (
            np.allclose(g, 1.0) and np.allclose(be, 0.0) and np.allclose(b2, 0.0)
        ), "general g/beta/b2 path not implemented"

    EF = edge_feat.shape[1] + r_feat.shape[1]  # 24
    # first-layer chunks (input rows: [ef 0:EF, dst EF:EF+D, src EF+D:EF+2D])
    w_ef_k = np.concatenate([w1k[:EF], b1k[None, :]], axis=0)  # [EF+1, 128]
    w_ef_v = np.concatenate([w1v[:EF], b1v[None, :]], axis=0)
    w_dst_k = w1k[EF : EF + D]
    w_dst_v = w1v[EF : EF + D]
    w_src_k = w1k[EF + D : EF + 2 * D]
    w_src_v = w1v[EF + D : EF + 2 * D]
    w2k = _f32(inputs["xk_W2"])
    w2v = _f32(inputs["xv_W2"])
    w2q = _f32(inputs["xq_W2"])
    w2o = _f32(inputs["out_W2"])
    w1oa = w1o[:D]
    w1oh = w1o[D : 2 * D]

    # head-indicator (scaled by 1/sqrt(hd)) zero-padded to 65 cols so the
    # score matmul writes the whole partition strip the evac reads (rows
    # 16-64 zeros; rows 32/64 later overwritten by the variance matmuls),
    # + a ones column (col 65) for the variance sums.
    hsel = np.zeros((D, 68), np.float32)
    for hh in range(HEADS):
        hsel[hh * hd : (hh + 1) * hd, hh] = 1.0 / np.sqrt(hd)
    hsel[:, 66] = 1.0

    n_per_core = (N + NCORES - 1) // NCORES
    B = (n_per_core + P - 1) // P + 3
    gblock_of, slot_of, core_of_g, lblock_of_g = _partition(dst, N, B)
    core_of_node = core_of_g[gblock_of]
    lblock_of_node = lblock_of_g[gblock_of]

    eg = gblock_of[dst]
    edge_order = np.argsort(eg, kind="stable")
    counts = np.bincount(eg[edge_order], minlength=NCORES * B)
    T = int((counts.max() + P - 1) // P)
    T = max(T, G)  # at least one full group
    starts = np.zeros(NCORES * B, np.int64)
    starts[1:] = np.cumsum(counts)[:-1]

    slots = np.full((NCORES, B * T * P), -1, np.int64)
    for g in range(NCORES * B):
        c = core_of_g[g]
        lb = lblock_of_g[g]
        cnt = counts[g]
        slots[c, lb * T * P : lb * T * P + cnt] = edge_order[
            starts[g] : starts[g] + cnt
        ]

    TOT = B * T * P
    efrfT = np.zeros((NCORES, EF + 1, TOT), np.float32)
    srci = np.zeros((NCORES, B, P, T), np.int32)
    ewf_a = np.zeros((NCORES, B, P, T), np.float32)
    S_a = np.zeros((NCORES, B, P, T, P), np.float32)  # [e-slot, t, node]
    for c in range(NCORES):
        s = slots[c]
        valid = s >= 0
        sv = s[valid]
        ef = np.concatenate([edge_feat[sv], r_feat[sv]], axis=1)
        efrfT[c, :EF, valid] = ef
        efrfT[c, EF, valid] = 1.0
        dloc = slot_of[dst[sv]].astype(np.int64)  # node slot within block
        # flat (b, t, p) -> (b, p, t)
        dn = np.full(TOT, -1, np.int64)
        dn[valid] = dloc
        dn = dn.reshape(B, T, P).transpose(0, 2, 1)  # [B, P, T]
        ew_col = np.zeros(TOT, np.float32)
        ew_col[valid] = e_w[sv]
        ewf_a[c] = ew_col.reshape(B, T, P).transpose(0, 2, 1)
        srcf = np.zeros(TOT, np.int64)
        srcf[valid] = src[sv]
        srci[c] = srcf.reshape(B, T, P).transpose(0, 2, 1)
        bi, pi, ti = np.nonzero(dn >= 0)
        S_a[c, bi, pi, ti, dn[bi, pi, ti]] = 1.0

    ST_a = S_a.transpose(0, 1, 4, 3, 2)  # [c, B, node, t, e-slot]

    hT = np.zeros((NCORES, D, B * P), np.float32)
    node_ids = np.arange(N)
    for c in range(NCORES):
        mask = core_of_node == c
        ids = node_ids[mask]
        pos = lblock_of_node[ids] * P + slot_of[ids]
        hT[c][:, pos] = h[ids].T

    hbf = _bf(h)
    in_maps = []
    for c in range(NCORES):
        in_maps.append(
            {
                "hrows": hbf,
                "hT": _bf(hT[c]),
                "efrfT": _bf(efrfT[c]),
                "srci": np.ascontiguousarray(srci[c]),
                "ew": _f32(ewf_a[c]),
                "S": _bf(S_a[c]),
                "ST": _bf(np.ascontiguousarray(ST_a[c])),
                "w_ef_k": _bf(w_ef_k),
                "w_ef_v": _bf(w_ef_v),
                "w_dst_k": _bf(w_dst_k),
                "w_dst_v": _bf(w_dst_v),
                "w_src_k": _bf(w_src_k),
                "w_src_v": _bf(w_src_v),
                "w2k": _bf(w2k),
                "w2v": _bf(w2v),
                "w1q": _bf(w1q),
                "w2q": _bf(w2q),
                "w1oa": _bf(w1oa),
                "w1oh": _bf(w1oh),
                "w2o": _bf(w2o),
                "hsel": _bf(hsel),
            }
        )

    meta = dict(
        N=N, D=D, E=E, B=B, T=T, EF=EF, hd=hd,
        core_of_node=core_of_node,
        lblock_of_node=lblock_of_node,
        slot_of=slot_of,
    )
    return meta, in_maps


# ---------------------------------------------------------------------------
def _build_graph(meta, debug=False, split_waits=True):
    N, D, B, T, EF = meta["N"], meta["D"], meta["B"], meta["T"], meta["EF"]
    hd = meta["hd"]
    TOT = B * T * P
    NG = (T + G - 1) // G  # groups per block

    nc = bass.Bass()
    hrows = nc.declare_dram_parameter("hrows", [N, D], BF16, isOutput=False)
    hT_d = nc.declare_dram_parameter("hT", [D, B * P], BF16, isOutput=False)
    efrfT_d = nc.declare_dram_parameter("efrfT", [EF + 1, TOT], BF16, isOutput=False)
    srci_d = nc.declare_dram_parameter("srci", [B, P, T], I32, isOutput=False)
    ew_d = nc.declare_dram_parameter("ew", [B, P, T], F32, isOutput=False)
    S_d = nc.declare_dram_parameter("S", [B, P, T, P], BF16, isOutput=False)
    ST_d = nc.declare_dram_parameter("ST", [B, P, T, P], BF16, isOutput=False)
    wnames = [
        ("w_ef_k", [EF + 1, D]),
        ("w_ef_v", [EF + 1, D]),
        ("w_dst_k", [D, D]),
        ("w_dst_v", [D, D]),
        ("w_src_k", [D, D]),
        ("w_src_v", [D, D]),
        ("w2k", [D, D]),
        ("w2v", [D, D]),
        ("w1q", [D, D]),
        ("w2q", [D, D]),
        ("w1oa", [D, D]),
        ("w1oh", [D, D]),
        ("w2o", [D, D]),
        ("hsel", [D, 68]),
    ]
    wd = {
        name: nc.declare_dram_parameter(name, shp, BF16, isOutput=False)
        for name, shp in wnames
    }
    out_d = nc.declare_dram_parameter("out", [B * P, D], F32, isOutput=True)
    if debug:
        dbg_q = nc.declare_dram_parameter("dbg_q", [B * P, D], F32, isOutput=True)
        dbg_sc = nc.declare_dram_parameter(
            "dbg_sc", [B, NG, 68, G * P], F32, isOutput=True)
        dbg_ct = nc.declare_dram_parameter("dbg_ct", [TOT, D + HEADS], F32, isOutput=True)
        dbg_acc = nc.declare_dram_parameter("dbg_acc", [B * P, D + HEADS], F32, isOutput=True)

    with TileContext(nc) as tc:
        with (
            tc.tile_pool(name="const", bufs=1) as cpool,
            tc.tile_pool(name="blk", bufs=2) as bpool,
            tc.tile_pool(name="grp", bufs=3) as epool,
            tc.tile_pool(name="ps_acc", bufs=1, space="PSUM") as ps_acc,
        ):
            # ---- constants ----
            W = {}
            for name, shp in wnames:
                t = cpool.tile(shp, BF16, tag="w_" + name, name="w_" + name)
                nc.sync.dma_start(out=t[:], in_=wd[name][:])
                W[name] = t
            ident = cpool.tile([P, P], BF16)
            make_identity(nc, ident[:])
            eps1 = cpool.tile([P, 1], F32)
            nc.gpsimd.memset(eps1[:], EPS)

            # persistent per-node-slot tables (all blocks)
            hTall = cpool.tile([P, B * P], BF16, tag="hTall")
            nc.sync.dma_start(out=hTall[:], in_=hT_d[:])
            qall = cpool.tile([P, B, P], BF16, tag="qall")
            adk_all = cpool.tile([P, B, P], BF16, tag="adk_all")
            adv_all = cpool.tile([P, B, P], BF16, tag="adv_all")

            def rstd_via_lnexp(var_ap, tag, pool):
                """rstd = exp(-0.5 * ln(var/D + EPS)) on ACT (one table set)."""
                lnv = pool.tile([P, 1], F32, tag="lnv_" + tag, name="lnv_" + tag)
                nc.scalar.activation(lnv[:], var_ap, AF.Ln,
                                     bias=eps1[:], scale=1.0 / D)
                rs = pool.tile([P, 1], F32, tag="rs_" + tag, name="rs_" + tag)
                nc.scalar.activation(rs[:], lnv[:], AF.Exp, scale=-0.5)
                return rs

            # ================= phase 0: per-block q-MLP + A_dst =============
            with tc.tile_pool(name="ps_p0", bufs=4, space="PSUM") as ps_p0:
                for b in range(B):
                    hTb = hTall[:, b * P : (b + 1) * P]
                    psA_k = ps_p0.tile([P, P], F32, tag="p0", name="psA_k")
                    nc.tensor.matmul(psA_k[:], lhsT=hTb, rhs=W["w_dst_k"][:],
                                     start=True, stop=True, skip_group_check=True)
                    nc.scalar.copy(out=adk_all[:, b, :], in_=psA_k[:])
                    psA_v = ps_p0.tile([P, P], F32, tag="p0", name="psA_v")
                    nc.tensor.matmul(psA_v[:], lhsT=hTb, rhs=W["w_dst_v"][:],
                                     start=True, stop=True, skip_group_check=True)
                    nc.scalar.copy(out=adv_all[:, b, :], in_=psA_v[:])
                    psQ = ps_p0.tile([P, P], F32, tag="p0", name="psQ")
                    nc.tensor.matmul(psQ[:], lhsT=hTb, rhs=W["w1q"][:],
                                     start=True, stop=True, skip_group_check=True)
                    varq = bpool.tile([P, 1], F32, tag="varq")
                    scrq = bpool.tile([P, D], BF16, tag="scrq")
                    nc.scalar.activation(scrq[:], psQ[:], AF.Square, accum_out=varq[:])
                    rstdq = rstd_via_lnexp(varq[:], "q", bpool)
                    hq = bpool.tile([P, D], BF16, tag="hq")
                    nc.vector.tensor_scalar_max(hq[:], psQ[:], 0.0)
                    hqT_ps = ps_p0.tile([P, P], BF16, tag="p0", name="hqT_ps")
                    nc.tensor.transpose(hqT_ps[:], hq[:], ident[:])
                    hqT = bpool.tile([P, P], BF16, tag="hqT")
                    nc.vector.tensor_copy(out=hqT[:], in_=hqT_ps[:])
                    psQ2 = ps_p0.tile([P, P], F32, tag="p0", name="psQ2")
                    nc.tensor.matmul(psQ2[:], lhsT=hqT[:], rhs=W["w2q"][:],
                                     start=True, stop=True, skip_group_check=True)
                    nc.vector.tensor_scalar_mul(qall[:, b, :], psQ2[:], rstdq[:])
                    if debug:
                        qf = bpool.tile([P, D], F32, tag="qf")
                        nc.vector.tensor_copy(out=qf[:], in_=qall[:, b, :])
                        nc.sync.dma_start(out=dbg_q[b * P : (b + 1) * P, :], in_=qf[:])

            # ================= main loop ====================================
            with (
                tc.tile_pool(name="ps_pre1", bufs=3, space="PSUM") as ps_pre1,
                tc.tile_pool(name="ps_rot", bufs=4, space="PSUM") as ps_rot,
                # PSUM: acc(1) + pre1(3) + rot(4) = 8 banks
            ):
                for b in range(B):
                    srcb = bpool.tile([P, T], I32, tag="srcb")
                    nc.sync.dma_start(out=srcb[:], in_=srci_d[b])
                    ewb = bpool.tile([P, T], F32, tag="ewb")
                    nc.sync.dma_start(out=ewb[:], in_=ew_d[b])
                    efb = bpool.tile([EF + 1, T * P], BF16, tag="efb")
                    nc.sync.dma_start(out=efb[:], in_=efrfT_d[:, b * T * P : (b + 1) * T * P])
                    Sb = bpool.tile([P, T, P], BF16, tag="Sb")
                    nc.sync.dma_start(out=Sb[:], in_=S_d[b])
                    STb = bpool.tile([P, T, P], BF16, tag="STb")
                    nc.sync.dma_start(out=STb[:], in_=ST_d[b])

                    acc = ps_acc.tile([P, D + HEADS], F32, tag="acc")

                    for g in range(NG):
                        t0 = g * G
                        gw = min(G, T - t0)  # subtiles in this group
                        W_ = gw * P  # edge width
                        ST_g = STb[:, t0 : t0 + gw, :]  # [n, gw, P] -> rhs
                        ef_g = efb[:, t0 * P : t0 * P + W_]

                        # ---- src-side gather ----
                        hs_g = epool.tile([P, 4, D], BF16, tag="hs_g")
                        for j in range(gw):
                            nc.gpsimd.indirect_dma_start(
                                out=hs_g[:, j, :], out_offset=None, in_=hrows[:],
                                in_offset=bass.IndirectOffsetOnAxis(
                                    ap=srcb[:, t0 + j : t0 + j + 1], axis=0),
                            )
                        hsT_ps = ps_rot.tile([P, 4, P], BF16, tag="rot", name="hsT_ps")
                        for j in range(gw):
                            nc.tensor.transpose(hsT_ps[:, j, :], hs_g[:, j, :], ident[:])
                        hsT = epool.tile([P, 4, P], BF16, tag="hsT")
                        nc.vector.tensor_copy(out=hsT[:, :gw, :], in_=hsT_ps[:, :gw, :])

                        # ---- q[dst] gather as matmul ----
                        qdT_ps = ps_rot.tile([P, 4 * P], F32, tag="rot", name="qdT_ps")
                        nc.tensor.matmul(qdT_ps[:, :W_], lhsT=qall[:, b, :], rhs=ST_g,
                                         start=True, stop=True, skip_group_check=True)
                        qdT = epool.tile([P, 4 * P], BF16, tag="qdT")
                        nc.scalar.copy(out=qdT[:, :W_], in_=qdT_ps[:, :W_])

                        # ---- first layer (transposed); dst chunk pre-folded --
                        pre1k = ps_pre1.tile([P, 4 * P], F32, tag="pre1", name="pre1k")
                        pre1v = ps_pre1.tile([P, 4 * P], F32, tag="pre1", name="pre1v")
                        for half, (pre1h, ad, wsrc, wef) in enumerate(
                            ((pre1k, adk_all, "w_src_k", "w_ef_k"),
                             (pre1v, adv_all, "w_src_v", "w_ef_v"))
                        ):
                            nc.tensor.matmul(pre1h[:, :W_], lhsT=ad[:, b, :],
                                             rhs=ST_g,
                                             start=True, stop=False,
                                             skip_group_check=True)
                            nc.tensor.matmul(pre1h[:, :W_], lhsT=W[wsrc][:],
                                             rhs=hsT[:, :gw, :],
                                             start=False, stop=False,
                                             skip_group_check=True)
                            nc.tensor.matmul(pre1h[:, :W_], lhsT=W[wef][:],
                                             rhs=ef_g,
                                             start=False, stop=True,
                                             skip_group_check=True)

                        # ---- square (for var) + relu ----
                        sq = epool.tile([P, 2, 4 * P], BF16, tag="sq")
                        nc.scalar.activation(sq[:, 0, :W_], pre1k[:, :W_], AF.Square)
                        nc.scalar.activation(sq[:, 1, :W_], pre1v[:, :W_], AF.Square)
                        hreluT = epool.tile([P, 2, 4 * P], BF16, tag="hreluT")
                        nc.vector.tensor_scalar_max(hreluT[:, 0, :W_], pre1k[:, :W_], 0.0)
                        nc.vector.tensor_scalar_max(hreluT[:, 1, :W_], pre1v[:, :W_], 0.0)

                        # ---- second layer: kT stays transposed ----
                        kT_ps = ps_rot.tile([P, 4 * P], F32, tag="rot", name="kT_ps")
                        nc.tensor.matmul(kT_ps[:, :W_], lhsT=W["w2k"][:],
                                         rhs=hreluT[:, 0, :W_],
                                         start=True, stop=True, skip_group_check=True)
                        prodT = epool.tile([P, 4 * P], BF16, tag="prodT")
                        nc.vector.tensor_tensor(
                            out=prodT[:, :W_], in0=kT_ps[:, :W_], in1=qdT[:, :W_],
                            op=ALU.mult,
                        )

                        # ---- scores (head-reduce) + variances in one bank ----
                        SCW = 65
                        scv = ps_rot.tile([P, 4 * P], F32, tag="rot", name="scv")
                        nc.tensor.matmul(scv[:SCW, :W_], lhsT=W["hsel"][:, :SCW],
                                         rhs=prodT[:, :W_],
                                         start=True, stop=True, skip_group_check=True)
                        nc.tensor.matmul(scv[32 : 33, :W_],
                                         lhsT=W["hsel"][:, 66 : 67],
                                         rhs=sq[:, 0, :W_],
                                         start=False, stop=True, skip_group_check=True)
                        nc.tensor.matmul(scv[64 : 65, :W_],
                                         lhsT=W["hsel"][:, 66 : 67],
                                         rhs=sq[:, 1, :W_],
                                         start=False, stop=True, skip_group_check=True)
                        sc_sb = epool.tile([SCW, 4 * P], BF16, tag="sc_sb")
                        nc.scalar.copy(out=sc_sb[:, :W_], in_=scv[:SCW, :W_])

                        # ---- back to edge-partition orientation ----
                        scn_ps = ps_rot.tile([P, 4, SCW + 1], BF16, tag="rot",
                                             name="scn_ps")
                        for j in range(gw):
                            nc.tensor.transpose(
                                scn_ps[:, j, :SCW], sc_sb[:, j * P : (j + 1) * P],
                                ident[:SCW, :SCW],
                            )
                        scores_n = epool.tile([P, 4, HEADS], BF16, tag="scores_n")
                        nc.vector.tensor_copy(out=scores_n[:, :gw, :],
                                              in_=scn_ps[:, :gw, :HEADS])
                        lnv = epool.tile([P, G, 2], F32, tag="lnv_g", name="lnv_g")
                        nc.scalar.activation(lnv[:, :gw, :], scn_ps[:, :gw, 32:65:32],
                                             AF.Ln, bias=eps1[:], scale=1.0 / D)
                        rstd = epool.tile([P, G, 2], F32, tag="rs_g", name="rs_g")
                        nc.scalar.activation(rstd[:, :gw, :], lnv[:, :gw, :],
                                             AF.Exp, scale=-0.5)

                        # ---- softmax numerator ----
                        scsc = epool.tile([P, 4, HEADS], BF16, tag="scsc")
                        nc.vector.tensor_tensor(
                            out=scsc[:, :gw, :], in0=scores_n[:, :gw, :],
                            in1=rstd[:, :gw, 0:1].to_broadcast([P, gw, HEADS]),
                            op=ALU.mult,
                        )
                        expn = epool.tile([P, 4, HEADS], BF16, tag="expn")
                        nc.scalar.activation(expn[:, :gw, :], scsc[:, :gw, :], AF.Exp)
                        rv = epool.tile([P, 4], F32, tag="rv")
                        nc.vector.tensor_tensor(
                            out=rv[:, :gw], in0=rstd[:, :gw, 1],
                            in1=ewb[:, t0 : t0 + gw], op=ALU.mult,
                        )
                        expn_s = epool.tile([P, 4, HEADS], BF16, tag="expn_s")
                        nc.vector.tensor_tensor(
                            out=expn_s[:, :gw, :], in0=expn[:, :gw, :],
                            in1=rv[:, :gw, None].to_broadcast([P, gw, HEADS]),
                            op=ALU.mult,
                        )

                        # ---- v (normal orientation) + contrib ----
                        v_ps = ps_rot.tile([P, 4, P], F32, tag="rot", name="v_ps")
                        for j in range(gw):
                            nc.tensor.matmul(v_ps[:, j, :],
                                             lhsT=hreluT[:, 1, j * P : (j + 1) * P],
                                             rhs=W["w2v"][:],
                                             start=True, stop=True,
                                             skip_group_check=True)
                        contrib = epool.tile([P, 4, D + HEADS], BF16, tag="contrib")
                        nc.vector.tensor_tensor(
                            out=contrib[:, :gw, :D].rearrange(
                                "p g (h d) -> p g h d", h=HEADS),
                            in0=expn_s[:, :gw, :, None].to_broadcast([P, gw, HEADS, hd]),
                            in1=v_ps[:, :gw, :].rearrange("p g (h d) -> p g h d", h=HEADS),
                            op=ALU.mult,
                        )
                        nc.vector.tensor_copy(out=contrib[:, :gw, D:],
                                              in_=expn[:, :gw, :])

                        if debug:
                            base = (b * T + t0) * P
                            scf = epool.tile([SCW, 4 * P], F32, tag="scf")
                            nc.vector.tensor_copy(out=scf[:, :W_], in_=scv[:SCW, :W_])
                            nc.sync.dma_start(out=dbg_sc[b, g, :SCW, :W_], in_=scf[:, :W_])
                            ctf = epool.tile([P, 4, D + HEADS], F32, tag="ctf")
                            nc.vector.tensor_copy(out=ctf[:, :gw, :], in_=contrib[:, :gw, :])
                            for j in range(gw):
                                nc.sync.dma_start(
                                    out=dbg_ct[base + j * P : base + (j + 1) * P, :],
                                    in_=ctf[:, j, :])

                        # ---- scatter ----
                        for j in range(gw):
                            nc.tensor.matmul(
                                acc[:], lhsT=Sb[:, t0 + j, :], rhs=contrib[:, j, :],
                                start=(g == 0 and j == 0),
                                stop=(g == NG - 1 and j == gw - 1),
                            )

                    # ---------- block epilogue ----------
                    if debug:
                        accf = bpool.tile([P, D + HEADS], F32, tag="accf")
                        nc.vector.tensor_copy(out=accf[:], in_=acc[:])
                        nc.sync.dma_start(out=dbg_acc[b * P : (b + 1) * P, :], in_=accf[:])
                    den_s = bpool.tile([P, HEADS], F32, tag="den_s")
                    nc.vector.tensor_scalar_add(den_s[:], acc[:, D:], 1e-30)
                    rden = bpool.tile([P, HEADS], F32, tag="rden")
                    nc.vector.reciprocal(rden[:], den_s[:])
                    attn = bpool.tile([P, D], BF16, tag="attn")
                    nc.vector.tensor_tensor(
                        out=attn[:].rearrange("p (h d) -> p h d", h=HEADS),
                        in0=acc[:, :D].rearrange("p (h d) -> p h d", h=HEADS),
                        in1=rden[:][:, :, None].to_broadcast([P, HEADS, hd]),
                        op=ALU.mult,
                    )
                    aT_ps = ps_rot.tile([P, 4, P], BF16, tag="rot", name="aT_ps")
                    nc.tensor.transpose(aT_ps[:, 0, :], attn[:], ident[:])
                    aT = bpool.tile([P, P], BF16, tag="aT")
                    nc.scalar.copy(out=aT[:], in_=aT_ps[:, 0, :])
                    psO = ps_rot.tile([P, 4 * P], F32, tag="rot", name="psO")
                    nc.tensor.matmul(psO[:, :P], lhsT=aT[:], rhs=W["w1oa"][:],
                                     start=True, stop=False)
                    nc.tensor.matmul(psO[:, :P], lhsT=hTall[:, b * P : (b + 1) * P],
                                     rhs=W["w1oh"][:],
                                     start=False, stop=True)
                    varo = bpool.tile([P, 1], F32, tag="varo")
                    scro = bpool.tile([P, D], BF16, tag="scro")
                    nc.scalar.activation(scro[:], psO[:, :P], AF.Square, accum_out=varo[:])
                    rsto = rstd_via_lnexp(varo[:], "o", bpool)
                    ho = bpool.tile([P, D], BF16, tag="ho")
                    nc.vector.tensor_scalar_max(ho[:], psO[:, :P], 0.0)
                    hoT_ps = ps_rot.tile([P, 4, P], BF16, tag="rot", name="hoT_ps")
                    nc.tensor.transpose(hoT_ps[:, 0, :], ho[:], ident[:])
                    hoT = bpool.tile([P, P], BF16, tag="hoT")
                    nc.scalar.copy(out=hoT[:], in_=hoT_ps[:, 0, :])
                    psO2 = ps_rot.tile([P, 4 * P], F32, tag="rot", name="psO2")
                    nc.tensor.matmul(psO2[:, :P], lhsT=hoT[:], rhs=W["w2o"][:],
                                     start=True, stop=True)
                    outb = bpool.tile([P, D], F32, tag="outb")
                    nc.vector.tensor_scalar_mul(outb[:], psO2[:, :P], rsto[:])
                    nc.sync.dma_start(out=out_d[b * P : (b + 1) * P, :], in_=outb[:])

    if split_waits:
        _split_excess_waits(nc)
    return nc


# ---------------------------------------------------------------------------
_CACHE = {}


def kernel(**inputs) -> np.ndarray:
    meta, in_maps = _prep(inputs)
    key = (meta["N"], meta["D"], meta["B"], meta["T"], meta["EF"])
    if key not in _CACHE:
        _CACHE[key] = _build_graph(meta)
    nc = _CACHE[key]

    res = run_bass_kernel_spmd(nc, in_maps, core_ids=list(range(NCORES)))
    N, D, B = meta["N"], meta["D"], meta["B"]
    out = np.empty((N, D), np.float32)
    pos = meta["lblock_of_node"] * P + meta["slot_of"]
    for c in range(NCORES):
        mask = meta["core_of_node"] == c
        out[mask] = res.results[c]["out"][pos[mask]]
    return out


# revision 4
# speedup vs baseline: 1.0586x; 1.0586x over previous
"""Trainium2 Bass kernel for BaseTopoLayer GNN message passing (v2).

Node-partitioned across 8 cores (each core owns all edges whose dst lands in
its node blocks; softmax segments fully local, no collectives).

v2 dataflow ("transposed hybrid"), per 512-edge group:
- First MLP layer runs channel-on-partition (transposed): pre1T[c,e] built
  from wide N=512 matmuls with the W1 chunks as stationary lhsT.
  h[dst]/q[dst] gathers are matmuls against a host-built selection matrix ST;
  h[src] rows are indirect-DMA gathered then PE-transposed.
- LayerNorm variance = ones-vector matmul over partitions (PE), so no ACT
  Square-accum per tile; rstd = exp(-.5 ln(var/D+eps)) stays in the single
  exp/ln/square/copy activation-table set.
- Scores: prodT = kT * qdstT elementwise, then a head-indicator matmul
  (1/sqrt(hd) folded in) reduces within heads; scores+vars ride one PSUM bank
  and are transposed back to edge-partition orientation together.
- Second layer consumes hreluT directly as lhsT (no transposes), producing
  k in transposed and v in normal orientation; softmax scales fold into the
  16 exp values per edge, not the 128 v values.
- Scatter per subtile is one matmul with the host-built S (= ST^T).
"""

import numpy as np
import ml_dtypes

import concourse.bass as bass
import concourse.mybir as mybir
from concourse.tile import TileContext
from concourse.vector_clock import ScopedClock
from concourse.bass_utils import run_bass_kernel_spmd
from concourse.masks import make_identity

BF16 = mybir.dt.bfloat16
F32 = mybir.dt.float32
I32 = mybir.dt.int32
AF = mybir.ActivationFunctionType
ALU = mybir.AluOpType

NCORES = 8
P = 128
HEADS = 16
EPS = 1e-5
G = 4  # subtiles (128 edges) per group


# ---------------------------------------------------------------------------
# Tile drain patch: this neuronxcc build rejects >N sem waits on one Drain.
def _patched_drain(self, tick_clock, wait_clock):
    nc = self.nc
    drain_inst = nc.sync.drain()
    wait_clock.add_sem_waits(
        drain_inst.ins, ScopedClock({None: tick_clock.global_clock})
    )
    si = drain_inst.ins.sync_info
    waits = list(si.on_wait or [])
    if len(waits) > 1:
        si.on_wait = [waits[0]]
        for w in waits[1:]:
            nop = nc.sync.nop(nofuse=True)
            nop.ins.sync_info = mybir.SyncInfo(on_wait=[w], on_update=[])
    nc.all_engine_barrier()
    assert self.sems is not None
    popped = nc._tile_sem_poison_stack.pop()
    assert popped is self._sem_poison
    nc.clear_and_free_semaphores(list(self.sems.allocated().values()))
    nc.all_engine_barrier()


TileContext._drain_and_barrier = _patched_drain


def _split_excess_waits(nc, max_waits=1):
    """Move excess sem waits onto same-engine nops placed just before."""
    cnt = 0
    for bb in nc.main_func.blocks:
        newlist = []
        for inst in bb.instructions:
            si = inst.sync_info
            waits = list(si.on_wait) if si is not None and si.on_wait else []
            if len(waits) > max_waits:
                si.on_wait = waits[:max_waits]
                for w in waits[max_waits:]:
                    nop = mybir.InstNoOp(name=f"waitnop-{cnt}", ins=[], outs=[])
                    cnt += 1
                    nop.engine = inst.engine
                    nop.sync_info = mybir.SyncInfo(on_wait=[w], on_update=[])
                    newlist.append(nop)
            newlist.append(inst)
        bb.instructions = newlist
    return cnt


def _bf(x):
    return np.ascontiguousarray(np.asarray(x, np.float32).astype(ml_dtypes.bfloat16))


def _f32(x):
    return np.ascontiguousarray(np.asarray(x, np.float32))


# ---------------------------------------------------------------------------
# Host-side partitioning: nodes -> (core, block, slot) with edge balancing.
def _partition(dst, N, B):
    import heapq

    G_ = NCORES * B
    deg = np.bincount(dst, minlength=N)
    order = np.argsort(-deg, kind="stable")
    heap = [(0, 0, g) for g in range(G_)]
    heapq.heapify(heap)
    gblock_of = np.empty(N, np.int32)
    slot_of = np.empty(N, np.int32)
    stash = []
    for n in order:
        while True:
            load, cnt, g = heapq.heappop(heap)
            if cnt < P:
                break
            stash.append((load, cnt, g))
        gblock_of[n] = g
        slot_of[n] = cnt
        heapq.heappush(heap, (load + int(deg[n]), cnt + 1, g))
        for s in stash:
            heapq.heappush(heap, s)
        stash.clear()
    loads = np.bincount(gblock_of, weights=deg, minlength=G_).astype(np.int64)
    order_g = np.argsort(-loads, kind="stable")
    core_of_g = np.empty(G_, np.int32)
    lblock_of_g = np.empty(G_, np.int32)
    core_loads = [(0.0, c) for c in range(NCORES)]
    heapq.heapify(core_loads)
    core_fill = [0] * NCORES
    for g in order_g:
        while True:
            cl, c = heapq.heappop(core_loads)
            if core_fill[c] < B:
                break
        core_of_g[g] = c
        lblock_of_g[g] = core_fill[c]
        core_fill[c] += 1
        heapq.heappush(core_loads, (cl + loads[g], c))
    return gblock_of, slot_of, core_of_g, lblock_of_g


# ---------------------------------------------------------------------------
def _prep(inputs):
    """All host-side preprocessing. Returns (meta, in_maps)."""
    h = _f32(inputs["h"])
    r_feat = _f32(inputs["r_feat"])
    edge_feat = _f32(inputs["edge_feat"])
    e_w = _f32(inputs["e_w"])
    ei = np.asarray(inputs["edge_index"])
    src = ei[0].astype(np.int64)
    dst = ei[1].astype(np.int64)

    N, D = h.shape
    E = src.shape[0]
    hd = D // HEADS
    assert D == 128, "kernel assumes D=128"

    def center(W1, b1):
        W1 = _f32(W1)
        b1 = _f32(b1)
        return W1 - W1.mean(axis=1, keepdims=True), b1 - b1.mean()

    w1k, b1k = center(inputs["xk_W1"], inputs["xk_b1"])
    w1v, b1v = center(inputs["xv_W1"], inputs["xv_b1"])
    w1q, b1q = center(inputs["xq_W1"], inputs["xq_b1"])
    w1o, b1o = center(inputs["out_W1"], inputs["out_b1"])
    assert np.allclose(b1q, 0.0) and np.allclose(b1o, 0.0), "nonzero b1 in q/out MLP"

    for m in ("xk", "xv", "xq", "out"):
        g = _f32(inputs[f"{m}_g"])
        be = _f32(inputs[f"{m}_beta"])
        b2 = _f32(inputs[f"{m}_b2"])
        assert (
            np.allclose(g, 1.0) and np.allclose(be, 0.0) and np.allclose(b2, 0.0)
        ), "general g/beta/b2 path not implemented"

    EF = edge_feat.shape[1] + r_feat.shape[1]  # 24
    # first-layer chunks (input rows: [ef 0:EF, dst EF:EF+D, src EF+D:EF+2D])
    w_ef_k = np.concatenate([w1k[:EF], b1k[None, :]], axis=0)  # [EF+1, 128]
    w_ef_v = np.concatenate([w1v[:EF], b1v[None, :]], axis=0)
    w_dst_k = w1k[EF : EF + D]
    w_dst_v = w1v[EF : EF + D]
    w_src_k = w1k[EF + D : EF + 2 * D]
    w_src_v = w1v[EF + D : EF + 2 * D]
    w2k = _f32(inputs["xk_W2"])
    w2v = _f32(inputs["xv_W2"])
    w2q = _f32(inputs["xq_W2"])
    w2o = _f32(inputs["out_W2"])
    w1oa = w1o[:D]
    w1oh = w1o[D : 2 * D]

    # head-indicator (scaled by 1/sqrt(hd)) zero-padded to 65 cols so the
    # score matmul writes the whole partition strip the evac reads (rows
    # 16-64 zeros; rows 32/64 later overwritten by the variance matmuls),
    # + a ones column (col 65) for the variance sums.
    hsel = np.zeros((D, 68), np.float32)
    for hh in range(HEADS):
        hsel[hh * hd : (hh + 1) * hd, hh] = 1.0 / np.sqrt(hd)
    hsel[:, 66] = 1.0

    n_per_core = (N + NCORES - 1) // NCORES
    B = (n_per_core + P - 1) // P + 3
    gblock_of, slot_of, core_of_g, lblock_of_g = _partition(dst, N, B)
    core_of_node = core_of_g[gblock_of]
    lblock_of_node = lblock_of_g[gblock_of]

    eg = gblock_of[dst]
    edge_order = np.argsort(eg, kind="stable")
    counts = np.bincount(eg[edge_order], minlength=NCORES * B)
    T = int((counts.max() + P - 1) // P)
    T = max(T, G)  # at least one full group
    starts = np.zeros(NCORES * B, np.int64)
    starts[1:] = np.cumsum(counts)[:-1]

    slots = np.full((NCORES, B * T * P), -1, np.int64)
    for g in range(NCORES * B):
        c = core_of_g[g]
        lb = lblock_of_g[g]
        cnt = counts[g]
        slots[c, lb * T * P : lb * T * P + cnt] = edge_order[
            starts[g] : starts[g] + cnt
        ]

    TOT = B * T * P
    efrfT = np.zeros((NCORES, EF + 1, TOT), np.float32)
    srci = np.zeros((NCORES, B, P, T), np.int32)
    ewf_a = np.zeros((NCORES, B, P, T), np.float32)
    S_a = np.zeros((NCORES, B, P, T, P), np.float32)  # [e-slot, t, node]
    for c in range(NCORES):
        s = slots[c]
        valid = s >= 0
        sv = s[valid]
        ef = np.concatenate([edge_feat[sv], r_feat[sv]], axis=1)
        efrfT[c, :EF, valid] = ef
        efrfT[c, EF, valid] = 1.0
        dloc = slot_of[dst[sv]].astype(np.int64)  # node slot within block
        # flat (b, t, p) -> (b, p, t)
        dn = np.full(TOT, -1, np.int64)
        dn[valid] = dloc
        dn = dn.reshape(B, T, P).transpose(0, 2, 1)  # [B, P, T]
        ew_col = np.zeros(TOT, np.float32)
        ew_col[valid] = e_w[sv]
        ewf_a[c] = ew_col.reshape(B, T, P).transpose(0, 2, 1)
        srcf = np.zeros(TOT, np.int64)
        srcf[valid] = src[sv]
        srci[c] = srcf.reshape(B, T, P).transpose(0, 2, 1)
        bi, pi, ti = np.nonzero(dn >= 0)
        S_a[c, bi, pi, ti, dn[bi, pi, ti]] = 1.0

    ST_a = S_a.transpose(0, 1, 4, 3, 2)  # [c, B, node, t, e-slot]

    hT = np.zeros((NCORES, D, B * P), np.float32)
    node_ids = np.arange(N)
    for c in range(NCORES):
        mask = core_of_node == c
        ids = node_ids[mask]
        pos = lblock_of_node[ids] * P + slot_of[ids]
        hT[c][:, pos] = h[ids].T

    hbf = _bf(h)
    in_maps = []
    for c in range(NCORES):
        in_maps.append(
            {
                "hrows": hbf,
                "hT": _bf(hT[c]),
                "efrfT": _bf(efrfT[c]),
                "srci": np.ascontiguousarray(srci[c]),
                "ew": _f32(ewf_a[c]),
                "S": _bf(S_a[c]),
                "ST": _bf(np.ascontiguousarray(ST_a[c])),
                "w_ef_k": _bf(w_ef_k),
                "w_ef_v": _bf(w_ef_v),
                "w_dst_k": _bf(w_dst_k),
                "w_dst_v": _bf(w_dst_v),
                "w_src_k": _bf(w_src_k),
                "w_src_v": _bf(w_src_v),
                "w2k": _bf(w2k),
                "w2v": _bf(w2v),
                "w1q": _bf(w1q),
                "w2q": _bf(w2q),
                "w1oa": _bf(w1oa),
                "w1oh": _bf(w1oh),
                "w2o": _bf(w2o),
                "hsel": _bf(hsel),
            }
        )

    meta = dict(
        N=N, D=D, E=E, B=B, T=T, EF=EF, hd=hd,
        core_of_node=core_of_node,
        lblock_of_node=lblock_of_node,
        slot_of=slot_of,
    )
    return meta, in_maps


# ---------------------------------------------------------------------------
def _build_graph(meta, debug=False, split_waits=True):
    N, D, B, T, EF = meta["N"], meta["D"], meta["B"], meta["T"], meta["EF"]
    hd = meta["hd"]
    TOT = B * T * P
    NG = (T + G - 1) // G  # groups per block

    nc = bass.Bass()
    hrows = nc.declare_dram_parameter("hrows", [N, D], BF16, isOutput=False)
    hT_d = nc.declare_dram_parameter("hT", [D, B * P], BF16, isOutput=False)
    efrfT_d = nc.declare_dram_parameter("efrfT", [EF + 1, TOT], BF16, isOutput=False)
    srci_d = nc.declare_dram_parameter("srci", [B, P, T], I32, isOutput=False)
    ew_d = nc.declare_dram_parameter("ew", [B, P, T], F32, isOutput=False)
    S_d = nc.declare_dram_parameter("S", [B, P, T, P], BF16, isOutput=False)
    ST_d = nc.declare_dram_parameter("ST", [B, P, T, P], BF16, isOutput=False)
    wnames = [
        ("w_ef_k", [EF + 1, D]),
        ("w_ef_v", [EF + 1, D]),
        ("w_dst_k", [D, D]),
        ("w_dst_v", [D, D]),
        ("w_src_k", [D, D]),
        ("w_src_v", [D, D]),
        ("w2k", [D, D]),
        ("w2v", [D, D]),
        ("w1q", [D, D]),
        ("w2q", [D, D]),
        ("w1oa", [D, D]),
        ("w1oh", [D, D]),
        ("w2o", [D, D]),
        ("hsel", [D, 68]),
    ]
    wd = {
        name: nc.declare_dram_parameter(name, shp, BF16, isOutput=False)
        for name, shp in wnames
    }
    out_d = nc.declare_dram_parameter("out", [B * P, D], F32, isOutput=True)
    if debug:
        dbg_q = nc.declare_dram_parameter("dbg_q", [B * P, D], F32, isOutput=True)
        dbg_sc = nc.declare_dram_parameter(
            "dbg_sc", [B, NG, 68, G * P], F32, isOutput=True)
        dbg_ct = nc.declare_dram_parameter("dbg_ct", [TOT, D + HEADS], F32, isOutput=True)
        dbg_acc = nc.declare_dram_parameter("dbg_acc", [B * P, D + HEADS], F32, isOutput=True)

    with TileContext(nc) as tc:
        with (
            tc.tile_pool(name="const", bufs=1) as cpool,
            tc.tile_pool(name="blk", bufs=3) as bpool,
            tc.tile_pool(name="grp", bufs=4) as epool,
            tc.tile_pool(name="ps_acc", bufs=1, space="PSUM") as ps_acc,
        ):
            # ---- constants ----
            W = {}
            for name, shp in wnames:
                t = cpool.tile(shp, BF16, tag="w_" + name, name="w_" + name)
                nc.sync.dma_start(out=t[:], in_=wd[name][:])
                W[name] = t
            ident = cpool.tile([P, P], BF16)
            make_identity(nc, ident[:])
            eps1 = cpool.tile([P, 1], F32)
            nc.gpsimd.memset(eps1[:], EPS)

            # persistent per-node-slot tables (all blocks)
            hTall = cpool.tile([P, B * P], BF16, tag="hTall")
            nc.sync.dma_start(out=hTall[:], in_=hT_d[:])
            qall = cpool.tile([P, B, P], BF16, tag="qall")
            adk_all = cpool.tile([P, B, P], BF16, tag="adk_all")
            adv_all = cpool.tile([P, B, P], BF16, tag="adv_all")

            def rstd_via_lnexp(var_ap, tag, pool):
                """rstd = exp(-0.5 * ln(var/D + EPS)) on ACT (one table set)."""
                lnv = pool.tile([P, 1], F32, tag="lnv_" + tag, name="lnv_" + tag)
                nc.scalar.activation(lnv[:], var_ap, AF.Ln,
                                     bias=eps1[:], scale=1.0 / D)
                rs = pool.tile([P, 1], F32, tag="rs_" + tag, name="rs_" + tag)
                nc.scalar.activation(rs[:], lnv[:], AF.Exp, scale=-0.5)
                return rs

            # ================= phase 0: per-block q-MLP + A_dst =============
            with tc.tile_pool(name="ps_p0", bufs=4, space="PSUM") as ps_p0:
                for b in range(B):
                    hTb = hTall[:, b * P : (b + 1) * P]
                    psA_k = ps_p0.tile([P, P], F32, tag="p0", name="psA_k")
                    nc.tensor.matmul(psA_k[:], lhsT=hTb, rhs=W["w_dst_k"][:],
                                     start=True, stop=True, skip_group_check=True)
                    nc.scalar.copy(out=adk_all[:, b, :], in_=psA_k[:])
                    psA_v = ps_p0.tile([P, P], F32, tag="p0", name="psA_v")
                    nc.tensor.matmul(psA_v[:], lhsT=hTb, rhs=W["w_dst_v"][:],
                                     start=True, stop=True, skip_group_check=True)
                    nc.scalar.copy(out=adv_all[:, b, :], in_=psA_v[:])
                    psQ = ps_p0.tile([P, P], F32, tag="p0", name="psQ")
                    nc.tensor.matmul(psQ[:], lhsT=hTb, rhs=W["w1q"][:],
                                     start=True, stop=True, skip_group_check=True)
                    varq = bpool.tile([P, 1], F32, tag="varq")
                    scrq = bpool.tile([P, D], BF16, tag="scrq")
                    nc.scalar.activation(scrq[:], psQ[:], AF.Square, accum_out=varq[:])
                    rstdq = rstd_via_lnexp(varq[:], "q", bpool)
                    hq = bpool.tile([P, D], BF16, tag="hq")
                    nc.vector.tensor_scalar_max(hq[:], psQ[:], 0.0)
                    hqT_ps = ps_p0.tile([P, P], BF16, tag="p0", name="hqT_ps")
                    nc.tensor.transpose(hqT_ps[:], hq[:], ident[:])
                    hqT = bpool.tile([P, P], BF16, tag="hqT")
                    nc.vector.tensor_copy(out=hqT[:], in_=hqT_ps[:])
                    psQ2 = ps_p0.tile([P, P], F32, tag="p0", name="psQ2")
                    nc.tensor.matmul(psQ2[:], lhsT=hqT[:], rhs=W["w2q"][:],
                                     start=True, stop=True, skip_group_check=True)
                    nc.vector.tensor_scalar_mul(qall[:, b, :], psQ2[:], rstdq[:])
                    if debug:
                        qf = bpool.tile([P, D], F32, tag="qf")
                        nc.vector.tensor_copy(out=qf[:], in_=qall[:, b, :])
                        nc.sync.dma_start(out=dbg_q[b * P : (b + 1) * P, :], in_=qf[:])

            # ================= main loop (3-stage skewed pipeline) ==========
            with (
                tc.tile_pool(name="ps_pre1", bufs=3, space="PSUM") as ps_pre1,
                tc.tile_pool(name="ps_rot", bufs=4, space="PSUM") as ps_rot,
                # PSUM: acc(1) + pre1(3) + rot(4) = 8 banks
            ):
                blkres = {}

                def load_block(b):
                    r = {}
                    r["srcb"] = bpool.tile([P, T], I32, tag="srcb", name="srcb")
                    nc.sync.dma_start(out=r["srcb"][:], in_=srci_d[b])
                    r["ewb"] = bpool.tile([P, T], F32, tag="ewb", name="ewb")
                    nc.sync.dma_start(out=r["ewb"][:], in_=ew_d[b])
                    r["efb"] = bpool.tile([EF + 1, T * P], BF16, tag="efb", name="efb")
                    nc.sync.dma_start(out=r["efb"][:],
                                      in_=efrfT_d[:, b * T * P : (b + 1) * T * P])
                    r["Sb"] = bpool.tile([P, T, P], BF16, tag="Sb", name="Sb")
                    nc.sync.dma_start(out=r["Sb"][:], in_=S_d[b])
                    r["STb"] = bpool.tile([P, T, P], BF16, tag="STb", name="STb")
                    nc.sync.dma_start(out=r["STb"][:], in_=ST_d[b])
                    return r

                def stage_a(w):
                    b, g = w["b"], w["g"]
                    t0, gw, W_ = w["t0"], w["gw"], w["W_"]
                    r = blkres[b]
                    ST_g = r["STb"][:, t0 : t0 + gw, :]
                    hs_g = epool.tile([P, 4, D], BF16, tag="hs_g", name="hs_g", bufs=8)
                    for j in range(gw):
                        nc.gpsimd.indirect_dma_start(
                            out=hs_g[:, j, :], out_offset=None, in_=hrows[:],
                            in_offset=bass.IndirectOffsetOnAxis(
                                ap=r["srcb"][:, t0 + j : t0 + j + 1], axis=0),
                        )
                    hsT_ps = ps_rot.tile([P, 4, P], BF16, tag="rot", name="hsT_ps")
                    for j in range(gw):
                        nc.tensor.transpose(hsT_ps[:, j, :], hs_g[:, j, :], ident[:])
                    hsT = epool.tile([P, 4, P], BF16, tag="hsT", name="hsT")
                    nc.vector.tensor_copy(out=hsT[:, :gw, :], in_=hsT_ps[:, :gw, :])

                    qdT_ps = ps_rot.tile([P, 4 * P], F32, tag="rot", name="qdT_ps")
                    nc.tensor.matmul(qdT_ps[:, :W_], lhsT=qall[:, b, :], rhs=ST_g,
                                     start=True, stop=True, skip_group_check=True)
                    qdT = epool.tile([P, 4 * P], BF16, tag="qdT", name="qdT")
                    nc.scalar.copy(out=qdT[:, :W_], in_=qdT_ps[:, :W_])

                    pre1k = ps_pre1.tile([P, 4 * P], F32, tag="pre1", name="pre1k")
                    pre1v = ps_pre1.tile([P, 4 * P], F32, tag="pre1", name="pre1v")
                    ef_g = r["efb"][:, t0 * P : t0 * P + W_]
                    for pre1h, ad, wsrc, wef in (
                        (pre1k, adk_all, "w_src_k", "w_ef_k"),
                        (pre1v, adv_all, "w_src_v", "w_ef_v"),
                    ):
                        nc.tensor.matmul(pre1h[:, :W_], lhsT=ad[:, b, :], rhs=ST_g,
                                         start=True, stop=False, skip_group_check=True)
                        nc.tensor.matmul(pre1h[:, :W_], lhsT=W[wsrc][:],
                                         rhs=hsT[:, :gw, :],
                                         start=False, stop=False, skip_group_check=True)
                        nc.tensor.matmul(pre1h[:, :W_], lhsT=W[wef][:], rhs=ef_g,
                                         start=False, stop=True, skip_group_check=True)
                    sq = epool.tile([P, 2, 4 * P], BF16, tag="sq", name="sq")
                    nc.scalar.activation(sq[:, 0, :W_], pre1k[:, :W_], AF.Square)
                    nc.scalar.activation(sq[:, 1, :W_], pre1v[:, :W_], AF.Square)
                    hreluT = epool.tile([P, 2, 4 * P], BF16, tag="hreluT", name="hreluT")
                    nc.vector.tensor_scalar_max(hreluT[:, 0, :W_], pre1k[:, :W_], 0.0)
                    nc.vector.tensor_scalar_max(hreluT[:, 1, :W_], pre1v[:, :W_], 0.0)
                    w["sq"], w["hreluT"], w["qdT"] = sq, hreluT, qdT

                def stage_b(w):
                    W_ = w["W_"]
                    sq, hreluT = w["sq"], w["hreluT"]
                    kT_ps = ps_rot.tile([P, 4 * P], F32, tag="rot", name="kT_ps")
                    nc.tensor.matmul(kT_ps[:, :W_], lhsT=W["w2k"][:],
                                     rhs=hreluT[:, 0, :W_],
                                     start=True, stop=True, skip_group_check=True)
                    prodT = epool.tile([P, 4 * P], BF16, tag="prodT", name="prodT")
                    nc.vector.tensor_tensor(
                        out=prodT[:, :W_], in0=kT_ps[:, :W_], in1=w["qdT"][:, :W_],
                        op=ALU.mult,
                    )
                    w["prodT"] = prodT

                def stage_c2(w):
                    W_ = w["W_"]
                    sq, prodT = w["sq"], w["prodT"]
                    SCW = 65
                    scv = ps_rot.tile([P, 4 * P], F32, tag="rot", name="scv")
                    nc.tensor.matmul(scv[:SCW, :W_], lhsT=W["hsel"][:, :SCW],
                                     rhs=prodT[:, :W_],
                                     start=True, stop=True, skip_group_check=True)
                    nc.tensor.matmul(scv[32 : 33, :W_], lhsT=W["hsel"][:, 66 : 67],
                                     rhs=sq[:, 0, :W_],
                                     start=False, stop=True, skip_group_check=True)
                    nc.tensor.matmul(scv[64 : 65, :W_], lhsT=W["hsel"][:, 66 : 67],
                                     rhs=sq[:, 1, :W_],
                                     start=False, stop=True, skip_group_check=True)
                    sc_sb = epool.tile([SCW, 4 * P], BF16, tag="sc_sb", name="sc_sb")
                    nc.scalar.copy(out=sc_sb[:, :W_], in_=scv[:SCW, :W_])
                    w["sc_sb"], w["scv"] = sc_sb, scv

                def stage_c(w):
                    b, g = w["b"], w["g"]
                    t0, gw, W_ = w["t0"], w["gw"], w["W_"]
                    r = blkres[b]
                    SCW = 65
                    sc_sb, hreluT = w["sc_sb"], w["hreluT"]
                    scn_ps = ps_rot.tile([P, 4, SCW + 1], BF16, tag="rot",
                                         name="scn_ps")
                    for j in range(gw):
                        nc.tensor.transpose(
                            scn_ps[:, j, :SCW], sc_sb[:, j * P : (j + 1) * P],
                            ident[:SCW, :SCW],
                        )
                    v_ps = ps_rot.tile([P, 4, P], F32, tag="rot", name="v_ps")
                    for j in range(gw):
                        nc.tensor.matmul(v_ps[:, j, :],
                                         lhsT=hreluT[:, 1, j * P : (j + 1) * P],
                                         rhs=W["w2v"][:],
                                         start=True, stop=True, skip_group_check=True)

                    scores_n = epool.tile([P, 4, HEADS], BF16, tag="scores_n",
                                          name="scores_n")
                    nc.vector.tensor_copy(out=scores_n[:, :gw, :],
                                          in_=scn_ps[:, :gw, :HEADS])
                    lnv = epool.tile([P, G, 2], F32, tag="lnv_g", name="lnv_g")
                    nc.scalar.activation(lnv[:, :gw, :], scn_ps[:, :gw, 32:65:32],
                                         AF.Ln, bias=eps1[:], scale=1.0 / D)
                    rstd = epool.tile([P, G, 2], F32, tag="rs_g", name="rs_g")
                    nc.scalar.activation(rstd[:, :gw, :], lnv[:, :gw, :],
                                         AF.Exp, scale=-0.5)
                    scsc = epool.tile([P, 4, HEADS], BF16, tag="scsc", name="scsc")
                    nc.vector.tensor_tensor(
                        out=scsc[:, :gw, :], in0=scores_n[:, :gw, :],
                        in1=rstd[:, :gw, 0:1].to_broadcast([P, gw, HEADS]),
                        op=ALU.mult,
                    )
                    expn = epool.tile([P, 4, HEADS], BF16, tag="expn", name="expn")
                    nc.scalar.activation(expn[:, :gw, :], scsc[:, :gw, :], AF.Exp)
                    rv = epool.tile([P, 4], F32, tag="rv", name="rv")
                    nc.vector.tensor_tensor(
                        out=rv[:, :gw], in0=rstd[:, :gw, 1],
                        in1=r["ewb"][:, t0 : t0 + gw], op=ALU.mult,
                    )
                    expn_s = epool.tile([P, 4, HEADS], BF16, tag="expn_s",
                                        name="expn_s")
                    nc.vector.tensor_tensor(
                        out=expn_s[:, :gw, :], in0=expn[:, :gw, :],
                        in1=rv[:, :gw, None].to_broadcast([P, gw, HEADS]),
                        op=ALU.mult,
                    )
                    contrib = epool.tile([P, 4, D + HEADS], BF16, tag="contrib",
                                         name="contrib")
                    nc.vector.tensor_tensor(
                        out=contrib[:, :gw, :D].rearrange(
                            "p g (h d) -> p g h d", h=HEADS),
                        in0=expn_s[:, :gw, :, None].to_broadcast([P, gw, HEADS, hd]),
                        in1=v_ps[:, :gw, :].rearrange("p g (h d) -> p g h d", h=HEADS),
                        op=ALU.mult,
                    )
                    nc.vector.tensor_copy(out=contrib[:, :gw, D:],
                                          in_=expn[:, :gw, :])

                    if debug:
                        base = (b * T + t0) * P
                        scf = epool.tile([SCW, 4 * P], F32, tag="scf", name="scf")
                        nc.vector.tensor_copy(out=scf[:, :W_], in_=w["scv"][:SCW, :W_])
                        nc.sync.dma_start(out=dbg_sc[b, g, :SCW, :W_], in_=scf[:, :W_])
                        ctf = epool.tile([P, 4, D + HEADS], F32, tag="ctf", name="ctf")
                        nc.vector.tensor_copy(out=ctf[:, :gw, :], in_=contrib[:, :gw, :])
                        for j in range(gw):
                            nc.sync.dma_start(
                                out=dbg_ct[base + j * P : base + (j + 1) * P, :],
                                in_=ctf[:, j, :])

                    w["contrib"] = contrib

                def stage_e(w):
                    b, g = w["b"], w["g"]
                    t0, gw = w["t0"], w["gw"]
                    r = blkres[b]
                    contrib = w["contrib"]
                    if g == 0:
                        blkres[b]["acc"] = ps_acc.tile([P, D + HEADS], F32, tag="acc",
                                                       name="acc")
                    acc = blkres[b]["acc"]
                    for j in range(gw):
                        nc.tensor.matmul(
                            acc[:], lhsT=r["Sb"][:, t0 + j, :], rhs=contrib[:, j, :],
                            start=(g == 0 and j == 0),
                            stop=(g == NG - 1 and j == gw - 1),
                        )

                def epilogue(b):
                    acc = blkres[b]["acc"]
                    if debug:
                        accf = bpool.tile([P, D + HEADS], F32, tag="accf", name="accf")
                        nc.vector.tensor_copy(out=accf[:], in_=acc[:])
                        nc.sync.dma_start(out=dbg_acc[b * P : (b + 1) * P, :], in_=accf[:])
                    den_s = bpool.tile([P, HEADS], F32, tag="den_s", name="den_s")
                    nc.vector.tensor_scalar_add(den_s[:], acc[:, D:], 1e-30)
                    rden = bpool.tile([P, HEADS], F32, tag="rden", name="rden")
                    nc.vector.reciprocal(rden[:], den_s[:])
                    attn = bpool.tile([P, D], BF16, tag="attn", name="attn")
                    nc.vector.tensor_tensor(
                        out=attn[:].rearrange("p (h d) -> p h d", h=HEADS),
                        in0=acc[:, :D].rearrange("p (h d) -> p h d", h=HEADS),
                        in1=rden[:][:, :, None].to_broadcast([P, HEADS, hd]),
                        op=ALU.mult,
                    )
                    aT_ps = ps_rot.tile([P, 4, P], BF16, tag="rot", name="aT_ps")
                    nc.tensor.transpose(aT_ps[:, 0, :], attn[:], ident[:])
                    aT = bpool.tile([P, P], BF16, tag="aT", name="aT")
                    nc.scalar.copy(out=aT[:], in_=aT_ps[:, 0, :])
                    psO = ps_rot.tile([P, 4 * P], F32, tag="rot", name="psO")
                    nc.tensor.matmul(psO[:, :P], lhsT=aT[:], rhs=W["w1oa"][:],
                                     start=True, stop=False)
                    nc.tensor.matmul(psO[:, :P], lhsT=hTall[:, b * P : (b + 1) * P],
                                     rhs=W["w1oh"][:],
                                     start=False, stop=True)
                    varo = bpool.tile([P, 1], F32, tag="varo", name="varo")
                    scro = bpool.tile([P, D], BF16, tag="scro", name="scro")
                    nc.scalar.activation(scro[:], psO[:, :P], AF.Square, accum_out=varo[:])
                    rsto = rstd_via_lnexp(varo[:], "o", bpool)
                    ho = bpool.tile([P, D], BF16, tag="ho", name="ho")
                    nc.vector.tensor_scalar_max(ho[:], psO[:, :P], 0.0)
                    hoT_ps = ps_rot.tile([P, 4, P], BF16, tag="rot", name="hoT_ps")
                    nc.tensor.transpose(hoT_ps[:, 0, :], ho[:], ident[:])
                    hoT = bpool.tile([P, P], BF16, tag="hoT", name="hoT")
                    nc.scalar.copy(out=hoT[:], in_=hoT_ps[:, 0, :])
                    psO2 = ps_rot.tile([P, 4 * P], F32, tag="rot", name="psO2")
                    nc.tensor.matmul(psO2[:, :P], lhsT=hoT[:], rhs=W["w2o"][:],
                                     start=True, stop=True)
                    outb = bpool.tile([P, D], F32, tag="outb", name="outb")
                    nc.vector.tensor_scalar_mul(outb[:], psO2[:, :P], rsto[:])
                    nc.sync.dma_start(out=out_d[b * P : (b + 1) * P, :], in_=outb[:])

                works = []
                for b in range(B):
                    for g in range(NG):
                        t0 = g * G
                        gw = min(G, T - t0)
                        works.append(dict(b=b, g=g, t0=t0, gw=gw, W_=gw * P))

                blkres[0] = load_block(0)
                n = len(works)

                def run_tail(i):
                    if i - 1 >= 0 and i - 1 < n:
                        stage_b(works[i - 1])
                    if i - 2 >= 0 and i - 2 < n:
                        stage_c2(works[i - 2])
                    if i - 3 >= 0 and i - 3 < n:
                        stage_c(works[i - 3])
                    if i - 4 >= 0 and i - 4 < n:
                        we = works[i - 4]
                        stage_e(we)
                        if we["g"] == NG - 1:
                            epilogue(we["b"])

                for i, w in enumerate(works):
                    if w["g"] == 0 and w["b"] + 1 < B:
                        blkres[w["b"] + 1] = load_block(w["b"] + 1)
                    stage_a(w)
                    run_tail(i)
                for i in range(n, n + 4):
                    run_tail(i)

    if split_waits:
        _split_excess_waits(nc)
    return nc


# ---------------------------------------------------------------------------
_CACHE = {}


def kernel(**inputs) -> np.ndarray:
    meta, in_maps = _prep(inputs)
    key = (meta["N"], meta["D"], meta["B"], meta["T"], meta["EF"])
    if key not in _CACHE:
        _CACHE[key] = _build_graph(meta)
    nc = _CACHE[key]

    res = run_bass_kernel_spmd(nc, in_maps, core_ids=list(range(NCORES)))
    N, D, B = meta["N"], meta["D"], meta["B"]
    out = np.empty((N, D), np.float32)
    pos = meta["lblock_of_node"] * P + meta["slot_of"]
    for c in range(NCORES):
        mask = meta["core_of_node"] == c
        out[mask] = res.results[c]["out"][pos[mask]]
    return out
